# revision 2
# baseline (speedup 1.0000x reference)
"""Trainium2 Bass kernel v3 for PhysicsInformedNN (Navier-Stokes PINN).

Same math as baseline (13-channel Taylor jet through 8-layer tanh MLP,
term-split accumulating matmuls), restructured for engine balance:

- Chains of W supertiles: all SBUF elementwise products run on wide
  [120, W*512] tiles, amortizing per-op fixed overhead.
- Every PSUM z-channel is staged to fp16 SBUF by the ACT engine right
  after its matmul group, freeing PSUM banks fast and keeping every DVE
  product in the cheap all-SBUF fp16 2x mode.
- scalar_tensor_tensor ops (1x mode) eliminated: constant factors 2/3
  are folded into scaled copies of the weight matrices.
- GpSimd left nearly idle (its tensor ops measured ~8x slower than DVE).
"""

import sys
from contextlib import ExitStack

import numpy as np

for _p in ("/opt/trn_rl_repo",):
    if _p not in sys.path:
        sys.path.insert(0, _p)

N_POINTS = 262144
N_CORES = 8
PPC = N_POINTS // N_CORES  # 32768
WIDTH = 20
N_HID = 7
G = 6
NPT = 512
SUPER = G * NPT            # 3072 points per supertile
NS = -(-PPC // SUPER)      # 11 supertiles per core
PADPC = NS * SUPER         # 33792
KDIM = G * WIDTH           # 120
M_OUT = 108
W = 2                      # supertiles per chain
CHAINS = []
_r = NS
while _r > 0:
    CHAINS.append(min(W, _r))
    _r -= min(W, _r)
NCH = len(CHAINS)
CW = W * NPT               # max chain width in columns

# channel ids
(VAL, CH_X, CH_Y, CH_T, CH_XX, CH_XY, CH_YY, CH_XT, CH_YT,
 CH_XXX, CH_XXY, CH_XYY, CH_YYY) = range(13)
DERIV_CHS = list(range(1, 13))
OUT1_CHS = [CH_X, CH_Y, CH_XX, CH_XY, CH_YY]
OUT2_CHS = [VAL, CH_X, CH_Y, CH_XT, CH_YT, CH_XXX, CH_XXY, CH_XYY, CH_YYY]


def build_host_consts(W_in, b_in, W_hid, b_hid, W_out, b_out, lb, ub,
                      lambda_1, lambda_2):
    f32 = np.float32
    W_in = np.asarray(W_in, f32)
    b_in = np.asarray(b_in, f32)
    W_hid = np.asarray(W_hid, f32)
    b_hid = np.asarray(b_hid, f32)
    W_out = np.asarray(W_out, f32)
    b_out = np.asarray(b_out, f32)
    lb = np.asarray(lb, f32)
    ub = np.asarray(ub, f32)

    s = (1.0 / (ub - lb)).astype(f32)
    c0 = (-lb * s).astype(f32)
    Wz = (W_in * s[:, None]).astype(f32)           # [3, 20]
    bz = (c0 @ W_in + b_in).astype(f32)            # [20]

    l1_lhsT = np.zeros((3 * G, KDIM), f32)
    hid_lhsT = np.zeros((N_HID, KDIM, KDIM), f32)
    for g in range(G):
        l1_lhsT[3 * g:3 * g + 3, WIDTH * g:WIDTH * (g + 1)] = Wz
        for l in range(N_HID):
            hid_lhsT[l, WIDTH * g:WIDTH * (g + 1),
                     WIDTH * g:WIDTH * (g + 1)] = W_hid[l]

    bias_tile = np.zeros((KDIM, 8), f32)
    bias_tile[:, 0] = np.tile(bz, G)
    for l in range(N_HID):
        bias_tile[:, 1 + l] = np.tile(b_hid[l], G)

    cx, cy, ct = Wz[0], Wz[1], Wz[2]
    cvecs = [cx, cy, ct,
             cx * cx, cx * cy, cy * cy, cx * ct, cy * ct,
             cx * cx * cx, cx * cx * cy, cx * cy * cy, cy * cy * cy]
    c_tile = np.stack([np.tile(v, G) for v in cvecs], axis=1).astype(f32)

    l1v = float(np.asarray(lambda_1).reshape(-1)[0])
    l2v = float(np.asarray(lambda_2).reshape(-1)[0])
    wpsi, wp = W_out[:, 0], W_out[:, 1]

    o1_lhsT = np.zeros((13, KDIM, M_OUT), f32)
    o2_lhsT = np.zeros((13, KDIM, M_OUT), f32)

    def place(arr, ch, row0, vec):
        for g in range(G):
            arr[ch, WIDTH * g:WIDTH * (g + 1), row0 + g] += vec

    place(o1_lhsT, CH_Y, 0, wpsi)        # u
    place(o1_lhsT, CH_Y, 6, wpsi)        # u
    place(o1_lhsT, CH_X, 32, -wpsi)      # v
    place(o1_lhsT, CH_X, 38, -wpsi)      # v
    place(o1_lhsT, CH_XY, 64, wpsi)      # psi_xy
    place(o1_lhsT, CH_XX, 70, wpsi)      # psi_xx
    place(o1_lhsT, CH_YY, 96, wpsi)      # psi_yy
    place(o1_lhsT, CH_XY, 102, wpsi)     # psi_xy

    place(o2_lhsT, CH_Y, 0, wpsi)        # u
    place(o2_lhsT, CH_X, 32, -wpsi)      # v
    place(o2_lhsT, VAL, 64, wp)          # p (bias added later)
    place(o2_lhsT, CH_YT, 96, wpsi)      # fu_lin
    place(o2_lhsT, CH_X, 96, wp)
    place(o2_lhsT, CH_XXY, 96, -l2v * wpsi)
    place(o2_lhsT, CH_YYY, 96, -l2v * wpsi)
    place(o2_lhsT, CH_XT, 102, -wpsi)    # fv_lin
    place(o2_lhsT, CH_Y, 102, wp)
    place(o2_lhsT, CH_XXX, 102, l2v * wpsi)
    place(o2_lhsT, CH_XYY, 102, l2v * wpsi)

    lam_vec = np.zeros((12, 1), f32)
    lam_vec[0:6, 0] = l1v
    lam_vec[6:12, 0] = -l1v

    f16 = np.float16
    return dict(l1_lhsT=l1_lhsT.astype(f16),
                hid_lhsT=hid_lhsT.astype(f16),
                hid2_lhsT=(2 * hid_lhsT).astype(f16),
                hid3_lhsT=(3 * hid_lhsT).astype(f16),
                bias_tile=bias_tile, c_tile=c_tile,
                o1_lhsT=o1_lhsT.astype(f16), o2_lhsT=o2_lhsT.astype(f16),
                o2_2_lhsT=(2 * o2_lhsT).astype(f16),
                o2_3_lhsT=(3 * o2_lhsT).astype(f16),
                lam_vec=lam_vec, p_bias=float(b_out[1]))


def build_program(p_bias):
    import concourse.bacc as bacc
    import concourse.bass as bass
    import concourse.tile as tile
    from concourse import mybir

    f32 = mybir.dt.float32
    f16 = mybir.dt.float16
    AF = mybir.ActivationFunctionType
    OP = mybir.AluOpType

    nc = bacc.Bacc("TRN2", target_bir_lowering=False, debug=False)

    xyz_d = nc.dram_tensor("xyz", [NCH, 3 * G, CW], f16, kind="ExternalInput")
    l1w_d = nc.dram_tensor("l1_lhsT", [3 * G, KDIM], f16, kind="ExternalInput")
    hw_d = nc.dram_tensor("hid_lhsT", [N_HID, KDIM, KDIM], f16,
                          kind="ExternalInput")
    hw2_d = nc.dram_tensor("hid2_lhsT", [N_HID, KDIM, KDIM], f16,
                           kind="ExternalInput")
    hw3_d = nc.dram_tensor("hid3_lhsT", [N_HID, KDIM, KDIM], f16,
                           kind="ExternalInput")
    o1_d = nc.dram_tensor("o1_lhsT", [13, KDIM, M_OUT], f16,
                          kind="ExternalInput")
    o2_d = nc.dram_tensor("o2_lhsT", [13, KDIM, M_OUT], f16,
                          kind="ExternalInput")
    o22_d = nc.dram_tensor("o2_2_lhsT", [13, KDIM, M_OUT], f16,
                           kind="ExternalInput")
    o23_d = nc.dram_tensor("o2_3_lhsT", [13, KDIM, M_OUT], f16,
                           kind="ExternalInput")
    bias_d = nc.dram_tensor("bias_tile", [KDIM, 8], f32, kind="ExternalInput")
    c_d = nc.dram_tensor("c_tile", [KDIM, 12], f32, kind="ExternalInput")
    lam_d = nc.dram_tensor("lam_vec", [12, 1], f32, kind="ExternalInput")
    u_d = nc.dram_tensor("u_out", [NS, G, NPT], f16, kind="ExternalOutput")
    v_d = nc.dram_tensor("v_out", [NS, G, NPT], f16, kind="ExternalOutput")
    p_d = nc.dram_tensor("p_out", [NS, G, NPT], f16, kind="ExternalOutput")
    fu_d = nc.dram_tensor("fu_out", [NS, G, NPT], f16, kind="ExternalOutput")
    fv_d = nc.dram_tensor("fv_out", [NS, G, NPT], f16, kind="ExternalOutput")

    with tile.TileContext(nc) as tc, ExitStack() as ctx:
        dma = nc.sync.dma_start
        act = nc.scalar.activation
        tt = nc.vector.tensor_tensor
        ts = nc.vector.tensor_scalar
        mm = nc.tensor.matmul

        # ---- persistent weights / constants ----
        wpool = ctx.enter_context(tc.tile_pool(name="wpool", bufs=1))
        l1w = wpool.tile([3 * G, KDIM], f16, name="l1w")
        dma(l1w[:], l1w_d[:])
        hw, hw2, hw3 = [], [], []
        for l in range(N_HID):
            for kind, (lst, src) in enumerate(
                    ((hw, hw_d), (hw2, hw2_d), (hw3, hw3_d))):
                w_ = wpool.tile([KDIM, KDIM], f16, name=f"hw{kind}_{l}")
                dma(w_[:], src[l])
                lst.append(w_)
        ow1, ow2, ow22, ow23 = {}, {}, {}, {}
        for ch in OUT1_CHS:
            w_ = wpool.tile([KDIM, M_OUT], f16, name=f"ow1_{ch}")
            dma(w_[:], o1_d[ch])
            ow1[ch] = w_
        for ch in OUT2_CHS:
            w_ = wpool.tile([KDIM, M_OUT], f16, name=f"ow2_{ch}")
            dma(w_[:], o2_d[ch])
            ow2[ch] = w_
        for ch in (CH_XXY, CH_XYY):
            w_ = wpool.tile([KDIM, M_OUT], f16, name=f"ow22_{ch}")
            dma(w_[:], o22_d[ch])
            ow22[ch] = w_
        for ch in (CH_XXX, CH_YYY):
            w_ = wpool.tile([KDIM, M_OUT], f16, name=f"ow23_{ch}")
            dma(w_[:], o23_d[ch])
            ow23[ch] = w_
        biases = wpool.tile([KDIM, 8], f32, name="biases")
        dma(biases[:], bias_d[:])
        ctile = wpool.tile([KDIM, 12], f32, name="ctile")
        dma(ctile[:], c_d[:])
        lam = wpool.tile([12, 1], f32, name="lam")
        dma(lam[:], lam_d[:])

        # ---- pools ----
        xin = ctx.enter_context(tc.tile_pool(name="xin", bufs=2))
        st2 = ctx.enter_context(tc.tile_pool(name="st2", bufs=2))
        st1 = ctx.enter_context(tc.tile_pool(name="st1", bufs=1))
        pp2 = ctx.enter_context(tc.tile_pool(name="pp2", bufs=2))
        pp1 = ctx.enter_context(tc.tile_pool(name="pp1", bufs=1))
        o12 = ctx.enter_context(tc.tile_pool(name="o12", bufs=2))
        o6 = ctx.enter_context(tc.tile_pool(name="o6", bufs=2))
        psz = ctx.enter_context(
            tc.tile_pool(name="psz", bufs=6, space=bass.MemorySpace.PSUM))
        pso = ctx.enter_context(
            tc.tile_pool(name="pso", bufs=2, space=bass.MemorySpace.PSUM))

        gtt = nc.gpsimd.tensor_tensor

        # group tiles: n blocks of CW columns each
        STAGE_SHAPES = {"T0": 1, "QG": 3, "SG2": 3}
        STAGE1_SHAPES = {"SGT": 2, "SG3": 4}
        PIECE_SHAPES = {"BG": 3, "VG2": 3, "VGT": 2, "TG3": 4, "CRG1": 2,
                        "CRG2": 2, "CRYY": 1, "GG1": 3, "GG2": 3,
                        "CUBG1": 2, "CUBG2": 2}
        TMP_SHAPES = {"P2": 1, "MNEG": 1, "QQ": 1, "F1": 1, "F2": 1,
                      "F3": 1, "PG": 2, "SSG": 2, "WG": 2}

        # stage block of each z-channel: (stage_tile, block)
        STAGE_OF = {CH_X: ("QG", 0), CH_Y: ("QG", 1), CH_T: ("QG", 2),
                    CH_XX: ("SG2", 0), CH_XY: ("SG2", 1), CH_YY: ("SG2", 2),
                    CH_XT: ("SGT", 0), CH_YT: ("SGT", 1),
                    CH_XXX: ("SG3", 0), CH_XXY: ("SG3", 1),
                    CH_XYY: ("SG3", 2), CH_YYY: ("SG3", 3)}

        def blk(tile_, i, cols):
            return tile_[:, i * CW:i * CW + cols]

        def view3(tile_, n, cols):
            return tile_[:, :n * CW].rearrange(
                "p (a c) -> p a c", a=n)[:, :, :cols]

        def bc(ap2d, n, cols):
            return ap2d.unsqueeze(1).broadcast_to([KDIM, n, cols])

        for c in range(NCH):
            wc = CHAINS[c]
            cols = wc * NPT
            cs = slice(0, cols)
            X = xin.tile([3 * G, CW], f16, name="X")
            dma(X[:, :cols], xyz_d[c, :, :cols])

            def alloc(shapes, pool):
                return {nm: pool.tile([KDIM, n * CW], f16, name=nm)
                        for nm, n in shapes.items()}

            S = alloc(STAGE_SHAPES, st2)
            S.update(alloc(STAGE1_SHAPES, st1))
            P = alloc(PIECE_SHAPES, pp2)
            T = alloc(TMP_SHAPES, pp1)

            # ---------- layer 1 ----------
            for s in range(wc):
                sb = slice(s * NPT, (s + 1) * NPT)
                pz = psz.tile([KDIM, NPT], f32, name="z", tag="z")
                mm(pz[:], l1w[:], X[:, sb], start=True, stop=True)
                act(S["T0"][:, sb], pz[:], AF.Tanh, bias=biases[:, 0:1])
            tt(T["P2"][:, cs], S["T0"][:, cs], S["T0"][:, cs], OP.mult)
            ts(T["MNEG"][:, cs], S["T0"][:, cs], -2.0, None, OP.mult)
            ts(T["QQ"][:, cs], T["P2"][:, cs], 6.0, -2.0, OP.mult, OP.add)
            ts(T["F1"][:, cs], T["P2"][:, cs], -1.0, 1.0, OP.mult, OP.add)
            tt(T["F2"][:, cs], T["MNEG"][:, cs], T["F1"][:, cs], OP.mult)
            tt(T["F3"][:, cs], T["QQ"][:, cs], T["F1"][:, cs], OP.mult)
            L1DST = {CH_X: ("BG", 0), CH_Y: ("BG", 1), CH_T: ("BG", 2),
                     CH_XX: ("VG2", 0), CH_XY: ("VG2", 1), CH_YY: ("VG2", 2),
                     CH_XT: ("VGT", 0), CH_YT: ("VGT", 1),
                     CH_XXX: ("TG3", 0), CH_XXY: ("TG3", 1),
                     CH_XYY: ("TG3", 2), CH_YYY: ("TG3", 3)}
            for k, ch in enumerate(DERIV_CHS):
                srcn = ("F1" if ch <= CH_T else
                        ("F2" if ch <= CH_YT else "F3"))
                dt_, di = L1DST[ch]
                ts(blk(P[dt_], di, cols), T[srcn][:, cs],
                   ctile[:, k:k + 1], None, OP.mult)
            pieces = {VAL: [("T0", 0, None)]}
            for ch in DERIV_CHS:
                dt_, di = L1DST[ch]
                pieces[ch] = [(dt_, di, None)]

            # ---------- hidden layers ----------
            for l in range(N_HID):
                Sp, S = S, alloc(STAGE_SHAPES, st2)
                S.update(alloc(STAGE1_SHAPES, st1))
                Pp, P = P, alloc(PIECE_SHAPES, pp2)
                T = alloc(TMP_SHAPES, pp1)

                def getp(nm, i, cs2, _sp=Sp, _pp=Pp):
                    t_ = _sp[nm] if nm == "T0" else _pp[nm]
                    return t_[:, i * CW + cs2.start:i * CW + cs2.stop]

                def mmch(ch, _s=S, _l=l, _pieces=pieces):
                    pl = _pieces[ch]
                    n = len(pl)
                    pzs = []
                    for s in range(wc):
                        sb = slice(s * NPT, (s + 1) * NPT)
                        pz = psz.tile([KDIM, NPT], f32, name="z", tag="z")
                        for i, (nm, bi, var) in enumerate(pl):
                            wv = (hw3[_l] if var == 3 else
                                  hw2[_l] if var == 2 else hw[_l])
                            mm(pz[:], wv[:], getp(nm, bi, sb),
                               start=(i == 0), stop=(i == n - 1))
                        pzs.append((pz, sb))
                    for pz, sb in pzs:
                        if ch == VAL:
                            act(_s["T0"][:, sb], pz[:], AF.Tanh,
                                bias=biases[:, 1 + _l:2 + _l])
                        else:
                            sn, si = STAGE_OF[ch]
                            act(_s[sn][:, si * CW + sb.start:
                                       si * CW + sb.stop], pz[:],
                                AF.Copy, bias=0.0, scale=1.0)

                mmch(VAL)
                mmch(CH_X)
                mmch(CH_Y)
                mmch(CH_T)
                tt(T["P2"][:, cs], S["T0"][:, cs], S["T0"][:, cs], OP.mult)
                ts(T["MNEG"][:, cs], S["T0"][:, cs], -2.0, None, OP.mult)
                ts(T["QQ"][:, cs], T["P2"][:, cs], 6.0, -2.0,
                   OP.mult, OP.add)
                ts(T["F1"][:, cs], T["P2"][:, cs], -1.0, 1.0,
                   OP.mult, OP.add)
                # fused products (broadcast middle dim keeps 2x mode)
                tt(view3(P["BG"], 3, cols), bc(T["F1"][:, cs], 3, cols),
                   view3(S["QG"], 3, cols), OP.mult)
                tt(view3(T["WG"], 2, cols), bc(T["MNEG"][:, cs], 2, cols),
                   view3(S["QG"], 2, cols), OP.mult)
                tt(view3(T["PG"], 2, cols), view3(P["BG"], 2, cols),
                   view3(S["QG"], 2, cols), OP.mult)
                tt(view3(T["SSG"], 2, cols), bc(T["QQ"][:, cs], 2, cols),
                   view3(T["PG"], 2, cols), OP.mult)
                tt(view3(P["CRG1"], 2, cols),
                   bc(blk(P["BG"], 0, cols), 2, cols),
                   view3(T["WG"], 2, cols), OP.mult)
                tt(blk(P["CRYY"], 0, cols), blk(P["BG"], 1, cols),
                   blk(T["WG"], 1, cols), OP.mult)
                tt(view3(P["CRG2"], 2, cols),
                   bc(blk(P["BG"], 2, cols), 2, cols),
                   view3(T["WG"], 2, cols), OP.mult)
                gtt(view3(P["CUBG1"], 2, cols),
                    bc(blk(T["SSG"], 0, cols), 2, cols),
                    view3(S["QG"], 2, cols), OP.mult)
                gtt(view3(P["CUBG2"], 2, cols),
                    bc(blk(T["SSG"], 1, cols), 2, cols),
                    view3(S["QG"], 2, cols), OP.mult)
                mmch(CH_XX)
                mmch(CH_XY)
                mmch(CH_YY)
                tt(view3(P["VG2"], 3, cols), bc(T["F1"][:, cs], 3, cols),
                   view3(S["SG2"], 3, cols), OP.mult)
                tt(view3(P["GG1"], 3, cols),
                   bc(blk(T["WG"], 0, cols), 3, cols),
                   view3(P["VG2"], 3, cols), OP.mult)
                tt(view3(P["GG2"], 3, cols),
                   bc(blk(T["WG"], 1, cols), 3, cols),
                   view3(P["VG2"], 3, cols), OP.mult)
                mmch(CH_XT)
                mmch(CH_YT)
                tt(view3(P["VGT"], 2, cols), bc(T["F1"][:, cs], 2, cols),
                   view3(S["SGT"], 2, cols), OP.mult)
                mmch(CH_XXX)
                mmch(CH_XXY)
                mmch(CH_XYY)
                mmch(CH_YYY)
                tt(view3(P["TG3"], 4, cols), bc(T["F1"][:, cs], 4, cols),
                   view3(S["SG3"], 4, cols), OP.mult)

                pieces = {
                    VAL: [("T0", 0, None)],
                    CH_X: [("BG", 0, None)], CH_Y: [("BG", 1, None)],
                    CH_T: [("BG", 2, None)],
                    CH_XX: [("VG2", 0, None), ("CRG1", 0, None)],
                    CH_XY: [("VG2", 1, None), ("CRG1", 1, None)],
                    CH_YY: [("VG2", 2, None), ("CRYY", 0, None)],
                    CH_XT: [("VGT", 0, None), ("CRG2", 0, None)],
                    CH_YT: [("VGT", 1, None), ("CRG2", 1, None)],
                    CH_XXX: [("TG3", 0, None), ("CUBG1", 0, None),
                             ("GG1", 0, 3)],
                    CH_XXY: [("TG3", 1, None), ("CUBG1", 1, None),
                             ("GG1", 1, 2), ("GG2", 0, None)],
                    CH_XYY: [("TG3", 2, None), ("CUBG2", 0, None),
                             ("GG2", 1, 2), ("GG1", 2, None)],
                    CH_YYY: [("TG3", 3, None), ("CUBG2", 1, None),
                             ("GG2", 2, 3)],
                }

            # ---------- output stage (per supertile) ----------
            for s in range(wc):
                sb = slice(s * NPT, (s + 1) * NPT)
                sidx = sum(CHAINS[:c]) + s

                def pslice(nm, bi, _s=S, _p=P, _sb=None):
                    t_ = _s[nm] if nm == "T0" else _p[nm]
                    return t_[:, bi * CW + _sb.start:bi * CW + _sb.stop]

                po1 = pso.tile([M_OUT, NPT], f32, name="po", tag="po")
                p1 = [(ch, nm, bi, var) for ch in OUT1_CHS
                      for (nm, bi, var) in pieces[ch]]
                for i, (ch, nm, bi, var) in enumerate(p1):
                    mm(po1[:], ow1[ch][:], pslice(nm, bi, _sb=sb),
                       start=(i == 0), stop=(i == len(p1) - 1))
                po2 = pso.tile([M_OUT, NPT], f32, name="po", tag="po")
                p2l = [(ch, nm, bi, var) for ch in OUT2_CHS
                       for (nm, bi, var) in pieces[ch]]
                for i, (ch, nm, bi, var) in enumerate(p2l):
                    wv = (ow23[ch] if var == 3 else
                          ow22[ch] if var == 2 else ow2[ch])
                    mm(po2[:], wv[:], pslice(nm, bi, _sb=sb),
                       start=(i == 0), stop=(i == len(p2l) - 1))
                a1 = o12.tile([12, NPT], f16, name="a1")
                act(a1[:], po1[0:12, :], AF.Identity, bias=0.0,
                    scale=lam[:, 0:1])
                a2 = o12.tile([12, NPT], f16, name="a2")
                act(a2[:], po1[32:44, :], AF.Identity, bias=0.0,
                    scale=lam[:, 0:1])
                pl1 = o12.tile([12, NPT], f16, name="pl1")
                tt(pl1[:], a1[:], po1[64:76, :], OP.mult)
                pl2 = o12.tile([12, NPT], f16, name="pl2")
                tt(pl2[:], a2[:], po1[96:108, :], OP.mult)
                dd = o12.tile([12, NPT], f16, name="dd")
                tt(dd[:], pl1[:], pl2[:], OP.add)
                ff = o12.tile([12, NPT], f16, name="ff")
                tt(ff[:], dd[:], po2[96:108, :], OP.add)
                pp = o6.tile([6, NPT], f16, name="pp")
                act(pp[:], po2[64:70, :], AF.Copy, bias=float(p_bias),
                    scale=1.0)
                uu = o6.tile([6, NPT], f16, name="uu")
                act(uu[:], po2[0:6, :], AF.Copy, bias=0.0, scale=1.0)
                vv = o6.tile([6, NPT], f16, name="vv")
                act(vv[:], po2[32:38, :], AF.Copy, bias=0.0, scale=1.0)

                dma(u_d[sidx], uu[:])
                dma(v_d[sidx], vv[:])
                dma(p_d[sidx], pp[:])
                dma(fu_d[sidx], ff[0:6, :])
                dma(fv_d[sidx], ff[6:12, :])

    nc.compile()
    return nc


def make_in_maps(inputs, consts):
    x = np.asarray(inputs["x"], np.float32).reshape(-1)
    y = np.asarray(inputs["y"], np.float32).reshape(-1)
    t = np.asarray(inputs["t"], np.float32).reshape(-1)
    shared = {k: consts[k] for k in
              ("l1_lhsT", "hid_lhsT", "hid2_lhsT", "hid3_lhsT", "o1_lhsT",
               "o2_lhsT", "o2_2_lhsT", "o2_3_lhsT", "bias_tile", "c_tile",
               "lam_vec")}
    in_maps = []
    for cpre in range(N_CORES):
        sl = slice(cpre * PPC, (cpre + 1) * PPC)

        def lay(vec):
            out = np.zeros((PADPC,), np.float32)
            seg = vec[sl]
            out[:seg.shape[0]] = seg[:PADPC]
            return out.reshape(NS, G, NPT)

        xyz = np.zeros((NS, 3 * G, NPT), np.float32)
        xyz[:, 0::3, :] = lay(x)
        xyz[:, 1::3, :] = lay(y)
        xyz[:, 2::3, :] = lay(t)
        # regroup supertiles into chains: [NCH, 18, CW]
        xyz_w = np.zeros((NCH, 3 * G, CW), np.float32)
        off = 0
        for ci, wc in enumerate(CHAINS):
            for s in range(wc):
                xyz_w[ci, :, s * NPT:(s + 1) * NPT] = xyz[off + s]
            off += wc
        in_maps.append({"xyz": xyz_w.astype(np.float16), **shared})
    return in_maps


def kernel(**inputs):
    consts = build_host_consts(
        inputs["W_in"], inputs["b_in"], inputs["W_hid"], inputs["b_hid"],
        inputs["W_out"], inputs["b_out"], inputs["lb"], inputs["ub"],
        inputs["lambda_1"], inputs["lambda_2"])
    nc = build_program(consts["p_bias"])
    in_maps = make_in_maps(inputs, consts)

    from concourse.bass_utils import run_bass_kernel_spmd
    res = run_bass_kernel_spmd(nc, in_maps, list(range(N_CORES)))

    outs = []
    for name in ("u_out", "v_out", "p_out", "fu_out", "fv_out"):
        full = np.concatenate(
            [np.asarray(res.results[ci][name]).reshape(-1)[:PPC]
             for ci in range(N_CORES)])
        outs.append(np.ascontiguousarray(full[:, None], dtype=np.float32))
    return tuple(outs)
